# revision 1
# baseline (speedup 1.0000x reference)
"""Trainium2 Bass kernel for nn_DPS_topk (topk_masking).

Math: the reference computes
    out = stop_gradient(hard - soft) + soft
which in the forward pass is numerically EXACTLY `hard` (the -soft and
+soft cancel bit-exactly in f32: positions where hard==0 give
(0 - s) + s == +0.0, and positions where hard==1 give (1 - s) + s which
rounds back to 1.0 for the tiny s produced here).  `hard` is the one-hot
expansion of the top-16 indices of (logits + gn) along D, with the k axis
ordered by ascending index.

Equivalently per row x (length D=1024):
    t   = 16th largest value of x
    m   = (x >= t)                         # membership mask, exactly 16 ones
    q   = inclusive_cumsum(m) * m          # rank 1..16 at selected, 0 else
    hard[j, d] = (q[d] == j + 1)

On-device algorithm per 128-row tile (rows on partitions, D on free axis):
    x   = gn_tile + logits_tile              (vector tensor_tensor add)
    v8  = max(x)                             (top-8 values, descending)
    x2  = match_replace(v8, x, -1e30)        (remove one instance of each)
    v16 = max(x2)                            (values ranked 9..16)
    m   = tensor_scalar(x, v16[:,7], is_ge)  (per-partition threshold)
    q   = tensor_tensor_scan(m, 0, add, add) (inclusive cumsum) * m
    out_j = tensor_scalar(q, j+1, is_equal)  for j in 0..15  -> (128, 16*1024)

Sharding: BS=32 split 4-per-core across 8 cores (data parallel); logits
replicated.  Per-core output (256, 16*1024) f32 = 16 MiB streamed as
32 x 512 KiB DMAs, one per computed one-hot plane, so the write stream
starts as soon as the first plane exists -> memory-bound at the
~360-400 GB/s HBM/core limit.

Raw Bass (no TileContext): this toolchain allows only ONE sync-wait
condition per instruction, which Tile's multi-wait tail drain violates.
Manual sems: every wait is a single wait_ge on a single semaphore.
Explicit vector.drain() between dependent same-engine DVE ops is
REQUIRED in raw Bass (verified on hardware: without them ~0.16% of
output elements are corrupted); the drains overlap the producer's
streaming phase so they only cost the ~0.3us pipe-empty tail.
"""

import numpy as np

K = 16
D = 1024
N = 64
BS = 32
NCORES = 8
BS_PER_CORE = BS // NCORES   # 4
ROWS = BS_PER_CORE * N       # 256 rows per core
P = 128                      # SBUF partitions
NTILES = ROWS // P           # 2

_CACHE = {}


def _build_nc():
    """Explicit DVE drains separate every dependent same-engine pair: raw
    Bass gets no automatic per-op pipeline drain (that is inserted by the
    Tile/bacc toolchain), and on hardware a dependent op issued before the
    producer's posted writes drain reads stale SBUF.  The drains overlap
    the producing op's streaming phase, so they cost only ~the pipe-empty
    tail (~0.3us) each."""
    import concourse.bass as bass
    from concourse import mybir

    f32 = mybir.dt.float32
    bf16 = mybir.dt.bfloat16
    A = mybir.AluOpType

    nc = bass.Bass()
    lg_d = nc.declare_dram_parameter("logits", [N, D], f32, isOutput=False)
    gn_d = nc.declare_dram_parameter("gn", [ROWS, D], f32, isOutput=False)
    out_d = nc.declare_dram_parameter("out", [ROWS, K * D], f32, isOutput=True)

    with (
        nc.sbuf_tensor([P, D], f32) as gt0,
        nc.sbuf_tensor([P, D], f32) as gt1,
        nc.sbuf_tensor([P, D], f32) as lg,
        nc.sbuf_tensor([P, D], f32) as zeros,
        nc.sbuf_tensor([P, D], f32) as x,
        nc.sbuf_tensor([P, 8], f32) as v8,
        nc.sbuf_tensor([P, D], f32) as x2,
        nc.sbuf_tensor([P, 8], f32) as v16,
        nc.sbuf_tensor([P, D], f32) as m,
        nc.sbuf_tensor([P, D], f32) as qi,
        nc.sbuf_tensor([P, D], f32) as q,
        nc.sbuf_tensor([P, K * D], f32) as chunk0,
        nc.sbuf_tensor([P, K * D], f32) as chunk1,
        nc.semaphore("in0_sem") as in0_sem,
        nc.semaphore("in0b_sem") as in0b_sem,
        nc.semaphore("in1_sem") as in1_sem,
        nc.semaphore("cmp_sem") as cmp_sem,
        nc.semaphore("dma_sem") as dma_sem,
        nc.Block(no_gpsimd_drain=True) as block,
    ):
        gts = [gt0, gt1]
        chunks = [chunk0, chunk1]

        @block.scalar
        def _(scalar: "bass.BassEngine"):
            # spread the three chain-0-gating input DMAs over three DMA
            # paths (SP HWDGE, ACT HWDGE, GpSimd SWDGE) so they transfer
            # in parallel
            scalar.dma_start(out=lg[0:N, :], in_=lg_d[:, :]).then_inc(in0_sem, 16)

        @block.gpsimd
        def _(gpsimd: "bass.BassEngine"):
            # SWDGE semaphores cannot be shared with HWDGE updaters
            gpsimd.dma_start(out=lg[N:P, :], in_=lg_d[:, :]).then_inc(in0b_sem, 16)

        @block.sync
        def _(sync: "bass.BassEngine"):
            # tile-0 gn first (gates chain-0 via in0_sem), tile-1 gn after
            sync.dma_start(out=gt0[:], in_=gn_d[0:P, :]).then_inc(in0_sem, 16)
            sync.dma_start(out=gt1[:], in_=gn_d[P : 2 * P, :]).then_inc(in1_sem, 16)

            # Stream each one-hot plane out as soon as it is SAFELY readable.
            # The eq op's sem-inc fires at instruction end, ~0.35us before its
            # posted writes finish draining, so gate plane j's DMA on eq j+1
            # (one 0.6us plane of slack); the final plane gates on the
            # per-tile drain whose inc fires only once the pipe is empty.
            SLACK = 1
            PER_TILE = K + 1  # 16 eq incs + 1 drain inc
            for i in range(NTILES):
                for j in range(K):
                    sync.wait_ge(
                        cmp_sem, PER_TILE * i + min(j + 1 + SLACK, PER_TILE)
                    )
                    sync.dma_start(
                        out=out_d[P * i : P * (i + 1), D * j : D * (j + 1)],
                        in_=chunks[i][:, D * j : D * (j + 1)],
                    ).then_inc(dma_sem, 16)

            # all output DMAs complete before kernel end
            sync.wait_ge(dma_sem, 16 * NTILES * K)

        @block.vector
        def _(vector: "bass.BassEngine"):
            def dr():
                vector.drain()

            vector.memset(zeros[:], 0.0)
            for i in range(NTILES):
                if i == 0:
                    vector.wait_ge(in0_sem, 32)
                    vector.wait_ge(in0b_sem, 16)
                else:
                    vector.wait_ge(in1_sem, 16)
                vector.tensor_tensor(x[:], gts[i][:], lg[:], op=A.add)
                dr()
                vector.max(v8[:], x[:])
                dr()
                vector.match_replace(x2[:], v8[:], x[:], -1e30)
                dr()
                vector.max(v16[:], x2[:])
                dr()
                vector.tensor_scalar(m[:], x[:], v16[:, 7:8], None, op0=A.is_ge)
                dr()
                vector.tensor_tensor_scan(
                    qi[:], m[:], zeros[:], 0.0, op0=A.add, op1=A.add
                )
                dr()
                vector.tensor_tensor(q[:], qi[:], m[:], op=A.mult)
                dr()
                for j in range(K):
                    vector.tensor_scalar(
                        chunks[i][:, D * j : D * (j + 1)],
                        q[:],
                        float(j + 1),
                        None,
                        op0=A.is_equal,
                    ).then_inc(cmp_sem, 1)
                # pipe-empty marker: gates the last SLACK planes' DMAs
                vector.drain().then_inc(cmp_sem, 1)

    return nc


def _get_nc():
    if "nc" not in _CACHE:
        _CACHE["nc"] = _build_nc()
    return _CACHE["nc"]


def kernel(logits: np.ndarray, gn: np.ndarray) -> np.ndarray:
    from concourse.bass_utils import run_bass_kernel_spmd

    logits = np.ascontiguousarray(np.asarray(logits, dtype=np.float32))
    gn = np.asarray(gn, dtype=np.float32)
    assert logits.shape == (N, D) and gn.shape == (BS, N, D)

    nc = _get_nc()
    in_maps = []
    for c in range(NCORES):
        shard = np.ascontiguousarray(
            gn[c * BS_PER_CORE : (c + 1) * BS_PER_CORE].reshape(ROWS, D)
        )
        in_maps.append({"logits": logits, "gn": shard})

    res = run_bass_kernel_spmd(nc, in_maps, list(range(NCORES))).results
    out = np.concatenate(
        [r["out"].reshape(BS_PER_CORE, N, K, D) for r in res], axis=0
    )
    return out.astype(np.float32, copy=False)



# revision 2
# speedup vs baseline: 1.3538x; 1.3538x over previous
"""Trainium2 Bass kernel for nn_DPS_topk (topk_masking).

Math: the reference computes out = stop_gradient(hard - soft) + soft,
which in the forward pass equals `hard` (exact cancellation in f32).
`hard` is the one-hot expansion along D of the top-16 indices of
(logits + gn), with the k axis ordered by ascending index.

Per row x (length D=1024):
    t   = 16th largest value of x            (max8 / match_replace8 / max8)
    m   = (x >= t)                           # 16 ones
    q   = inclusive_cumsum(m) * m            # rank 1..16 at selected, 0 else
    hard[j, d] = (q[d] == j + 1)

v2 design (evidence from the v1 trace):
  * v1 was limited by (a) f32 DVE throughput for the 16 one-hot planes
    (~610 ns each at 2x) and the f32 chain, (b) 16.8 MiB of f32 HBM
    writes (~42 us at the ~400 GB/s streaming rate), (c) a 12 us DMA
    idle gap before the first plane, (d) an 8 us tail from DMA-engine
    15 lagging.
  * The one-hot values are exactly {0, 1}: compute the rank tensor q in
    fp16 (integers <= 16 are exact; the scan accumulates in fp32
    internally) so the 16 is_eq plane ops run at DVE 4x (16-bit) mode,
    and write the planes as uint8 -- the DRAM output is uint8 and the
    host widens to f32 (lossless, values are exact 0/1).  HBM write
    traffic drops 4x to 4.2 MiB/core.
  * Selection (add/max8/match_replace/max8/is_ge) stays in f32 --
    ordering decisions need full precision.
  * Output DMAs are grouped (4/4/4/2/2 planes) to amortize the ~1.2 us
    HWDGE issue cost, and gated with 2 planes of slack (a plane's
    sem-inc fires ~0.35 us before its posted writes drain; 2 later
    planes ~0.78 us cover that).  The last group of each tile gates on
    the pipe-empty drain marker.
  * Inputs: gn tile0 on the sync HWDGE ring; logits (both partition
    halves) and gn tile1 on the scalar HWDGE ring, so the tile-0
    critical path only waits for 0.75 MiB.

Raw Bass (no TileContext): one sync-wait condition per instruction;
explicit vector.drain() between dependent same-engine DVE ops is
REQUIRED (verified on hardware) -- the drains overlap the producer's
streaming phase so they cost only the pipe-empty tail.
"""

import numpy as np

K = 16
D = 1024
N = 64
BS = 32
NCORES = 8
BS_PER_CORE = BS // NCORES   # 4
ROWS = BS_PER_CORE * N       # 256 rows per core
P = 128                      # SBUF partitions
NTILES = ROWS // P           # 2

# plane-group layout for output DMAs: (start_plane, n_planes)
GROUPS = [(0, 4), (4, 4), (8, 4), (12, 2), (14, 2)]
SLACK = 2                    # planes of production slack before group DMA
PER_TILE = K + 1             # 16 plane incs + 1 drain inc on cmp_sem

_CACHE = {}


def _build_nc():
    import concourse.bass as bass
    from concourse import mybir

    f32 = mybir.dt.float32
    f16 = mybir.dt.float16
    u8 = mybir.dt.uint8
    A = mybir.AluOpType

    nc = bass.Bass()
    lg_d = nc.declare_dram_parameter("logits", [N, D], f32, isOutput=False)
    gn_d = nc.declare_dram_parameter("gn", [ROWS, D], f32, isOutput=False)
    out_d = nc.declare_dram_parameter("out", [ROWS, K * D], u8, isOutput=True)

    with (
        nc.sbuf_tensor([P, D], f32) as gt0,
        nc.sbuf_tensor([P, D], f32) as gt1,
        nc.sbuf_tensor([P, D], f32) as lg,
        nc.sbuf_tensor([P, D], f32) as x,
        nc.sbuf_tensor([P, D], f32) as x2,
        nc.sbuf_tensor([P, 8], f32) as v8,
        nc.sbuf_tensor([P, 8], f32) as v16,
        nc.sbuf_tensor([P, D], f16) as m,
        nc.sbuf_tensor([P, D], f16) as qi,
        nc.sbuf_tensor([P, D], f16) as q0,
        nc.sbuf_tensor([P, D], f16) as q1,
        nc.sbuf_tensor([P, K * D], u8) as chunk0,
        nc.sbuf_tensor([P, K * D], u8) as chunk1,
        nc.semaphore("in0_sem") as in0_sem,
        nc.semaphore("in1_sem") as in1_sem,
        nc.semaphore("cmp_sem") as cmp_sem,
        nc.semaphore("dma_sem") as dma_sem,
        nc.Block(no_gpsimd_drain=True) as block,
    ):
        gts = [gt0, gt1]
        qs = [q0, q1]
        chunks = [chunk0, chunk1]

        @block.scalar
        def _(scalar: "bass.BassEngine"):
            # logits replicated into both partition halves + gn tile 1,
            # all on the ACT HWDGE ring (sync ring carries only gn tile 0
            # so the tile-0 critical path is short)
            scalar.dma_start(out=lg[0:N, :], in_=lg_d[:, :]).then_inc(in0_sem, 16)
            scalar.dma_start(out=lg[N:P, :], in_=lg_d[:, :]).then_inc(in0_sem, 16)
            scalar.dma_start(out=gt1[:], in_=gn_d[P : 2 * P, :]).then_inc(in1_sem, 16)

        @block.sync
        def _(sync: "bass.BassEngine"):
            sync.dma_start(out=gt0[:], in_=gn_d[0:P, :]).then_inc(in0_sem, 16)

            for i in range(NTILES):
                for s, l in GROUPS:
                    gate = PER_TILE * i + min(s + l + SLACK, PER_TILE)
                    sync.wait_ge(cmp_sem, gate)
                    sync.dma_start(
                        out=out_d[P * i : P * (i + 1), D * s : D * (s + l)],
                        in_=chunks[i][:, D * s : D * (s + l)],
                    ).then_inc(dma_sem, 16)

            sync.wait_ge(dma_sem, 16 * NTILES * len(GROUPS))

        @block.vector
        def _(vector: "bass.BassEngine"):
            def dr():
                vector.drain()

            for i in range(NTILES):
                if i == 0:
                    vector.wait_ge(in0_sem, 48)
                else:
                    vector.wait_ge(in1_sem, 16)
                vector.tensor_tensor(x[:], gts[i][:], lg[:], op=A.add)
                dr()
                vector.max(v8[:], x[:])
                dr()
                vector.match_replace(x2[:], v8[:], x[:], -1e30)
                dr()
                vector.max(v16[:], x2[:])
                dr()
                # m: fp16 membership mask (0/1 exact)
                vector.tensor_scalar(m[:], x[:], v16[:, 7:8], None, op0=A.is_ge)
                dr()
                # inclusive cumsum of m -> rank at selected positions;
                # fp32 internal accumulator, fp16 output (ints <= 16 exact)
                vector.tensor_tensor_scan(
                    qi[:], m[:], m[:], 0.0, op0=A.add, op1=A.bypass
                )
                dr()
                vector.tensor_tensor(qs[i][:], qi[:], m[:], op=A.mult)
                dr()
                for j in range(K):
                    vector.tensor_scalar(
                        chunks[i][:, D * j : D * (j + 1)],
                        qs[i][:],
                        float(j + 1),
                        None,
                        op0=A.is_equal,
                    ).then_inc(cmp_sem, 1)
                # pipe-empty marker gates the last SLACK planes' DMAs
                vector.drain().then_inc(cmp_sem, 1)

    return nc


def _get_nc():
    if "nc" not in _CACHE:
        _CACHE["nc"] = _build_nc()
    return _CACHE["nc"]


def kernel(logits: np.ndarray, gn: np.ndarray) -> np.ndarray:
    from concourse.bass_utils import run_bass_kernel_spmd

    logits = np.ascontiguousarray(np.asarray(logits, dtype=np.float32))
    gn = np.asarray(gn, dtype=np.float32)
    assert logits.shape == (N, D) and gn.shape == (BS, N, D)

    nc = _get_nc()
    in_maps = []
    for c in range(NCORES):
        shard = np.ascontiguousarray(
            gn[c * BS_PER_CORE : (c + 1) * BS_PER_CORE].reshape(ROWS, D)
        )
        in_maps.append({"logits": logits, "gn": shard})

    res = run_bass_kernel_spmd(nc, in_maps, list(range(NCORES))).results
    out = np.concatenate(
        [r["out"].reshape(BS_PER_CORE, N, K, D) for r in res], axis=0
    )
    # device output is uint8 holding exact {0,1}; widen losslessly
    return out.astype(np.float32)


# revision 3
# speedup vs baseline: 1.4976x; 1.1062x over previous
"""Trainium2 Bass kernel for nn_DPS_topk (topk_masking).

Math: the reference computes out = stop_gradient(hard - soft) + soft,
which in the forward pass equals `hard` (exact cancellation in f32).
`hard` is the one-hot expansion along D of the top-16 indices of
(logits + gn), with the k axis ordered by ascending index.

Per row x (length D=1024):
    t   = 16th largest value of x            (max8 / match_replace8 / max8)
    m   = (x >= t)                           # 16 ones
    q   = inclusive_cumsum(m) * m            # rank 1..16 at selected, 0 else
    hard[j, d] = (q[d] == j + 1)

v3 design (evidence from v1/v2 traces):
  * One-hot values are exactly {0,1}: rank tensor q is fp16 (ints <= 16
    exact; scan accumulates fp32 internally), the 16 is_eq plane ops run
    in DVE 4x mode (~330 ns vs 686 ns) writing fp16 planes, and the
    DRAM output is fp16 -- the host widens to f32 (lossless).  HBM
    writes halve to 8.4 MiB/core (~21 us at the measured ~400 GB/s).
  * Selection (add/max8/match_replace/max8/is_ge) stays in f32.
  * Input latency: the three input loads ride three different DMA rings
    (gn tile0 on sync HWDGE, logits on scalar HWDGE) and gn tile0 is
    column-split so the tensor_tensor add starts on the first half
    while the second lands (v2 lost ~2.3 us to one serialized ring).
  * Tile 1's add runs as a SWDGE accumulate-DMA (gn tile1 += onto the
    logits buffer, issued by gpsimd after the tile-0 add releases it),
    removing 1.2 us from the DVE serial path.
  * Output plane-group DMAs: small leading groups (1/1/2 planes) start
    the write stream ~1 us after q exists; 4-plane body groups amortize
    the HWDGE issue cost; 1-plane final groups on BOTH rings shrink the
    drain tail.  Groups gate on the plane-completion semaphore with 2
    planes of slack (a plane's sem-inc fires ~0.35 us before its posted
    writes drain); each tile's final groups gate on the pipe-empty
    drain marker.

Raw Bass (no TileContext): one sync-wait condition per instruction;
explicit vector.drain() between dependent same-engine DVE ops is
REQUIRED (verified on hardware) -- they overlap the producer's
streaming phase and only cost the pipe-empty tail.
"""

import numpy as np

K = 16
D = 1024
N = 64
BS = 32
NCORES = 8
BS_PER_CORE = BS // NCORES   # 4
ROWS = BS_PER_CORE * N       # 256 rows per core
P = 128                      # SBUF partitions
NTILES = ROWS // P           # 2

# plane-group layout for output DMAs: (start_plane, n_planes)
GROUPS = [(0, 1), (1, 1), (2, 2), (4, 4), (8, 4), (12, 2), (14, 1), (15, 1)]
SLACK = 2                    # planes of production slack before group DMA
PER_TILE = K + 1             # 16 plane incs + 1 drain inc on cmp_sem

_CACHE = {}


def _gate(i, s, l):
    return PER_TILE * i + min(s + l + SLACK, PER_TILE)


def _build_nc():
    import concourse.bass as bass
    from concourse import mybir

    f32 = mybir.dt.float32
    f16 = mybir.dt.float16
    A = mybir.AluOpType
    H = D // 2

    nc = bass.Bass()
    lg_d = nc.declare_dram_parameter("logits", [N, D], f32, isOutput=False)
    gn_d = nc.declare_dram_parameter("gn", [ROWS, D], f32, isOutput=False)
    out_d = nc.declare_dram_parameter("out", [ROWS, K * D], f16, isOutput=True)

    with (
        nc.sbuf_tensor([P, D], f32) as gt0,
        nc.sbuf_tensor([P, D], f32) as lg,   # logits replicated; later becomes x1 = lg + gn1
        nc.sbuf_tensor([P, D], f32) as x,    # tile-0 perturbed logits
        nc.sbuf_tensor([P, D], f32) as x2,
        nc.sbuf_tensor([P, 8], f32) as v8,
        nc.sbuf_tensor([P, 8], f32) as v16,
        nc.sbuf_tensor([P, D], f16) as m,
        nc.sbuf_tensor([P, D], f16) as qi,
        nc.sbuf_tensor([P, D], f16) as q0,
        nc.sbuf_tensor([P, D], f16) as q1,
        nc.sbuf_tensor([P, K * D], f16) as chunk0,
        nc.sbuf_tensor([P, K * D], f16) as chunk1,
        nc.semaphore("in0a_sem") as in0a_sem,   # gn0 left half + logits
        nc.semaphore("in0b_sem") as in0b_sem,   # gn0 right half
        nc.semaphore("xfree_sem") as xfree_sem, # DVE add done -> lg reusable
        nc.semaphore("in1_sem") as in1_sem,     # SWDGE accum gn1 done
        nc.semaphore("cmp_sem") as cmp_sem,
        nc.semaphore("dma_sem") as dma_sem,
        nc.Block(no_gpsimd_drain=True) as block,
    ):
        xs = [x, lg]
        qs = [q0, q1]
        chunks = [chunk0, chunk1]

        # ring assignment for output groups: tile0 -> sync, tile1 -> scalar,
        # except tile1's (15,1) goes to sync so the two final 1-plane DMAs
        # issue in parallel on different rings.
        sync_groups = [(0, s, l) for (s, l) in GROUPS] + [(1, 15, 1)]
        scalar_groups = [(1, s, l) for (s, l) in GROUPS[:-1]]

        @block.scalar
        def _(scalar: "bass.BassEngine"):
            scalar.dma_start(out=lg[0:N, :], in_=lg_d[:, :]).then_inc(in0a_sem, 16)
            scalar.dma_start(out=lg[N:P, :], in_=lg_d[:, :]).then_inc(in0a_sem, 16)
            for i, s, l in scalar_groups:
                scalar.wait_ge(cmp_sem, _gate(i, s, l))
                scalar.dma_start(
                    out=out_d[P * i : P * (i + 1), D * s : D * (s + l)],
                    in_=chunks[i][:, D * s : D * (s + l)],
                ).then_inc(dma_sem, 16)

        @block.gpsimd
        def _(gpsimd: "bass.BassEngine"):
            # tile-1 add for free: accumulate gn rows 128..255 onto the
            # replicated logits once the tile-0 add has consumed them
            gpsimd.wait_ge(xfree_sem, 1)
            gpsimd.dma_start(
                out=lg[:, :], in_=gn_d[P : 2 * P, :], accum_op=A.add
            ).then_inc(in1_sem, 16)

        @block.sync
        def _(sync: "bass.BassEngine"):
            sync.dma_start(out=gt0[:, 0:H], in_=gn_d[0:P, 0:H]).then_inc(
                in0a_sem, 16
            )
            sync.dma_start(out=gt0[:, H:D], in_=gn_d[0:P, H:D]).then_inc(
                in0b_sem, 16
            )
            for i, s, l in sync_groups:
                sync.wait_ge(cmp_sem, _gate(i, s, l))
                sync.dma_start(
                    out=out_d[P * i : P * (i + 1), D * s : D * (s + l)],
                    in_=chunks[i][:, D * s : D * (s + l)],
                ).then_inc(dma_sem, 16)
            sync.wait_ge(dma_sem, 16 * (len(sync_groups) + len(scalar_groups)))

        @block.vector
        def _(vector: "bass.BassEngine"):
            def dr():
                vector.drain()

            for i in range(NTILES):
                if i == 0:
                    # add in column halves so work starts as soon as the
                    # first half of gn tile0 lands
                    vector.wait_ge(in0a_sem, 48)
                    vector.tensor_tensor(
                        x[:, 0:H], gt0[:, 0:H], lg[:, 0:H], op=A.add
                    )
                    vector.wait_ge(in0b_sem, 16)
                    vector.tensor_tensor(
                        x[:, H:D], gt0[:, H:D], lg[:, H:D], op=A.add
                    ).then_inc(xfree_sem, 1)
                    dr()
                else:
                    # x1 = lg + gn1 was produced by the SWDGE accum DMA
                    vector.wait_ge(in1_sem, 16)
                vector.max(v8[:], xs[i][:])
                dr()
                vector.match_replace(x2[:], v8[:], xs[i][:], -1e30)
                dr()
                vector.max(v16[:], x2[:])
                dr()
                vector.tensor_scalar(
                    m[:], xs[i][:], v16[:, 7:8], None, op0=A.is_ge
                )
                dr()
                vector.tensor_tensor_scan(
                    qi[:], m[:], m[:], 0.0, op0=A.add, op1=A.bypass
                )
                dr()
                vector.tensor_tensor(qs[i][:], qi[:], m[:], op=A.mult)
                dr()
                for j in range(K):
                    vector.tensor_scalar(
                        chunks[i][:, D * j : D * (j + 1)],
                        qs[i][:],
                        float(j + 1),
                        None,
                        op0=A.is_equal,
                    ).then_inc(cmp_sem, 1)
                vector.drain().then_inc(cmp_sem, 1)

    return nc


def _get_nc():
    if "nc" not in _CACHE:
        _CACHE["nc"] = _build_nc()
    return _CACHE["nc"]


def kernel(logits: np.ndarray, gn: np.ndarray) -> np.ndarray:
    from concourse.bass_utils import run_bass_kernel_spmd

    logits = np.ascontiguousarray(np.asarray(logits, dtype=np.float32))
    gn = np.asarray(gn, dtype=np.float32)
    assert logits.shape == (N, D) and gn.shape == (BS, N, D)

    nc = _get_nc()
    in_maps = []
    for c in range(NCORES):
        shard = np.ascontiguousarray(
            gn[c * BS_PER_CORE : (c + 1) * BS_PER_CORE].reshape(ROWS, D)
        )
        in_maps.append({"logits": logits, "gn": shard})

    res = run_bass_kernel_spmd(nc, in_maps, list(range(NCORES))).results
    out = np.concatenate(
        [r["out"].reshape(BS_PER_CORE, N, K, D) for r in res], axis=0
    )
    # device output is fp16 holding exact {0,1}; widen losslessly
    return out.astype(np.float32)
